# revision 15
# baseline (speedup 1.0000x reference)
"""Multi-head attention (QK-LayerNorm, causal) Trainium2 kernel over 8 NeuronCores.

Sharding: tensor-parallel over heads - 2 heads per core. Each core computes
q/k/v projections for its 128 channels, per-head attention for both batches,
and a partial output projection (its 128-channel slice of Wo); the host sums
the 8 partial projections.

Key layout/throughput choices (vs the f32r baseline):
- All DRAM traffic and matmul operands are bf16 (PE rate is identical to
  fp32r at large free-dims, but DMA bytes halve and small-free matmuls avoid
  the fp32r 4x penalty). PSUM accumulation stays f32.
- x is pre-tiled on the host into the exact SBUF layout, so each input tile
  is one DMA of 128 contiguous 2KB rows (4x fewer descriptors).
- Scores for both heads of a (b, qc, kt) step go into one 2-bank PSUM tile
  [128, 1024]; a single Exp drains both, halving ACT access overhead.
- LayerNorm mean-subtraction is folded into the weights on the host; rstd is
  exp(-0.5*ln(var+eps)) (Exp/Ln share an ACT table; Sqrt does not).
- The softmax denominator is the 65th row of attn@v via a ones-column in V.
- Everything is software-pipelined against the in-order engine streams:
  projection tiles interleave with attention chunks (PSUM pools are shared,
  not phase-scoped), q/k transposes trail their tile by two projection
  steps, attn@v trails exp by one kt step, and the out-projection of chunk
  i drains inside the kt-loop of chunk i+1.
- Engine balance: Square/qkT-drain/rstd on ACT, PSUM-draining reductions,
  multiplies and copies on DVE, diagonal masking on Pool (no PSUM port).
"""

import numpy as np
import ml_dtypes

import concourse.bass as bass
import concourse.mybir as mybir
import concourse.tile as tile
from concourse.bass_utils import run_bass_kernel_spmd
from concourse.masks import make_identity

F32 = mybir.dt.float32
F32R = mybir.dt.float32r
BF16 = mybir.dt.bfloat16
BF16NP = ml_dtypes.bfloat16

B, S, D, H = 2, 2048, 1024, 16
DH = D // H          # 64
NCORES = 8
HPC = H // NCORES    # 2 heads per core
CH = HPC * DH        # 128 channels per core
T = B * S            # 4096 tokens
DCH = D // 128       # 8 contraction chunks
TT = T // 128        # 32 token tiles
QW = 512             # q-chunk width
QC = S // QW         # 4 q-chunks per batch
KTB = S // 128       # 16 k-tiles per batch
EPS = 1e-5


def _split_drain_waits(nc):
    """walrus in this env only accepts one sync-wait per instruction;
    hoist extra waits onto preceding single-wait NOPs on the same engine."""
    for f in nc.m.functions:
        for blk in f.blocks:
            new_insts = []
            for inst in blk.instructions:
                si = getattr(inst, "sync_info", None)
                if si is not None and si.on_wait and len(si.on_wait) > 1:
                    waits = list(si.on_wait)
                    for j, w in enumerate(waits[:-1]):
                        new_insts.append(
                            mybir.InstNoOp(
                                name=f"{inst.name}-dwsplit{j}",
                                engine=inst.engine,
                                ins=[],
                                outs=[],
                                sync_info=mybir.SyncInfo(on_wait=[w], on_update=[]),
                            )
                        )
                    si.on_wait = [waits[-1]]
                    inst.sync_info = si
                new_insts.append(inst)
            blk.instructions[:] = new_insts


def _build(use_bias=False):
    nc = bass.Bass("TRN2", target_bir_lowering=False, debug=False)

    # x pre-tiled on host: row (128*t + p) holds x^T[(a*128+p), 128*t:128*(t+1)]
    # flattened over a, i.e. the SBUF tile layout for token-tile t.
    xtl_d = nc.dram_tensor("xtl", [T, D], BF16, kind="ExternalInput")
    wqkvt_d = nc.dram_tensor("wqkvt", [D, 3 * CH], BF16, kind="ExternalInput")
    bqkv_d = (
        nc.dram_tensor("bqkv", [1, 3 * CH], F32, kind="ExternalInput")
        if use_bias
        else None
    )
    wot_d = nc.dram_tensor("wot", [CH, D], BF16, kind="ExternalInput")
    pot_d = nc.dram_tensor("pot", [D, T], BF16, kind="ExternalOutput")

    AF = mybir.ActivationFunctionType
    ALU = mybir.AluOpType

    with tile.TileContext(nc) as tc:
        with (
            tc.tile_pool(name="const", bufs=1) as const_pool,
            tc.tile_pool(name="big", bufs=1) as big,
            tc.tile_pool(name="xt", bufs=6) as xpool,
            tc.tile_pool(name="sq", bufs=3) as sq_pool,
            tc.tile_pool(name="ln", bufs=4) as ln_pool,
            tc.tile_pool(name="qln", bufs=5) as qln_pool,
            tc.tile_pool(name="qkv", bufs=3) as qkv_pool,
            tc.tile_pool(name="ex", bufs=4) as ex_pool,
            tc.tile_pool(name="ao", bufs=3) as ao_pool,
            tc.tile_pool(name="dr", bufs=3) as dr_pool,
            tc.tile_pool(name="po", bufs=6) as po_pool,
            tc.tile_pool(name="psS", bufs=2, space="PSUM") as psS,
            tc.tile_pool(name="psM", bufs=2, space="PSUM") as psM,
            tc.tile_pool(name="psO", bufs=2, space="PSUM") as psO,
        ):
            ident_f = const_pool.tile([128, 128], F32)
            make_identity(nc, ident_f)
            ident = const_pool.tile([128, 128], BF16)
            nc.vector.tensor_copy(out=ident, in_=ident_f)

            epscol = const_pool.tile([128, 1], F32)
            nc.vector.memset(epscol, EPS)

            ones64f = const_pool.tile([1, DH], F32)
            nc.vector.memset(ones64f, 1.0)
            ones64 = const_pool.tile([1, DH], F32R)
            nc.vector.tensor_copy(out=ones64, in_=ones64f)

            wqkv_sb = const_pool.tile([128, DCH, 3 * CH], BF16)
            for d in range(DCH):
                nc.sync.dma_start(
                    out=wqkv_sb[:, d, :],
                    in_=wqkvt_d[128 * d : 128 * (d + 1), :],
                )
            wo_sb = const_pool.tile([128, D], BF16)
            nc.sync.dma_start(out=wo_sb, in_=wot_d[:, :])
            if use_bias:
                bias_sb = const_pool.tile([128, 3 * CH], F32)
                nc.sync.dma_start(
                    out=bias_sb, in_=bqkv_d[0:1, :].to_broadcast([128, 3 * CH])
                )

            # persistent activations
            qkT = big.tile([128, 2, T], BF16)     # [:,0,:]=q^T  [:,1,:]=k^T
            vaug = big.tile([128, TT, 2 * (DH + 1)], BF16)
            ones32 = const_pool.tile([128, TT, 1], BF16)
            nc.vector.memset(ones32, 1.0)
            for h in range(HPC):
                oc = (DH + 1) * h + DH
                nc.vector.tensor_copy(out=vaug[:, :, oc : oc + 1], in_=ones32)

            trq = []       # lagged transpose+drain closures, one per tile
            po_pending = []  # lagged out-projection steps

            def flush_po():
                if po_pending:
                    po_pending.pop(0)()

            def emit_tile(t):
                xt_sb = xpool.tile([128, DCH, 128], BF16, tag="xt")
                xin = xtl_d[128 * t : 128 * (t + 1), :].rearrange(
                    "p (a j) -> p a j", j=128
                )
                nsplit = 4 if t < 2 else 1
                step = DCH // nsplit
                for part in range(nsplit):
                    nc.sync.dma_start(
                        out=xt_sb[:, part * step : (part + 1) * step, :],
                        in_=xin[:, part * step : (part + 1) * step, :],
                    )
                ps = psM.tile([128, 3 * CH], F32, tag="m")
                for d in range(DCH):
                    nc.tensor.matmul(
                        ps,
                        lhsT=xt_sb[:, d, :],
                        rhs=wqkv_sb[:, d, :],
                        start=(d == 0),
                        stop=(d == DCH - 1),
                    )
                if use_bias:
                    qkv = qkv_pool.tile([128, 3 * CH], F32, tag="qkv")
                    nc.vector.tensor_add(out=qkv, in0=ps, in1=bias_sb)
                    src = qkv
                else:
                    src = ps

                # rstd = exp(-0.5*ln(mean(q'^2) + eps)) per (token, head)
                sq = sq_pool.tile([128, 2 * CH], BF16, tag="sq")
                nc.scalar.activation(out=sq, in_=src[:, 0 : 2 * CH], func=AF.Square)
                ssum = ln_pool.tile([128, 4], F32, tag="ssum")
                nc.vector.reduce_sum(
                    out=ssum,
                    in_=sq.rearrange("p (g x) -> p g x", x=DH),
                    axis=mybir.AxisListType.X,
                )
                lnv = ln_pool.tile([128, 4], F32, tag="lnv")
                nc.scalar.activation(
                    out=lnv, in_=ssum, func=AF.Ln, scale=1.0 / DH,
                    bias=epscol[:, :],
                )
                rstd = ln_pool.tile([128, 4], F32, tag="rstd")
                nc.scalar.activation(out=rstd, in_=lnv, func=AF.Exp, scale=-0.5)

                qln = qln_pool.tile([128, 2 * CH], BF16, tag="qln")
                rstd_ap = rstd[:, :]
                rstd_b = bass.AP(
                    tensor=rstd_ap.tensor,
                    offset=rstd_ap.offset,
                    ap=rstd_ap.ap + [[0, DH]],
                )
                nc.vector.tensor_mul(
                    out=qln.rearrange("p (g x) -> p g x", x=DH),
                    in0=src[:, 0 : 2 * CH].rearrange("p (g x) -> p g x", x=DH),
                    in1=rstd_b,
                )
                nc.vector.tensor_copy(
                    out=vaug[:, t, :].rearrange("p (h x) -> p h x", x=DH + 1)[
                        :, :, 0:DH
                    ],
                    in_=src[:, 2 * CH : 3 * CH].rearrange("p (h x) -> p h x", x=DH),
                )

                def transp(t=t, qln=qln):
                    pst = psM.tile([128, 256], BF16, tag="m")
                    nc.tensor.transpose(pst[:, 0:128], qln[:, 0:CH], ident)
                    nc.tensor.transpose(pst[:, 128:256], qln[:, CH : 2 * CH], ident)
                    nc.scalar.copy(
                        out=qkT[:, :, 128 * t : 128 * (t + 1)],
                        in_=pst[:, :].rearrange("p (i x) -> p i x", x=128),
                    )

                trq.append(transp)
                if len(trq) > 2:
                    trq.pop(0)()

            def emit_chunk(b, qc):
                q0 = b * S + qc * QW
                n_kt = (qc + 1) * (QW // 128)
                ao = ao_pool.tile([128, QW], BF16, tag="ao")
                pso0 = psO.tile([DH + 1, QW], F32, tag="o")
                pso1 = psO.tile([DH + 1, QW], F32, tag="o")
                pso = [pso0, pso1]
                pending_av = None
                for kt in range(n_kt):
                    c0 = max(0, 128 * kt - qc * QW)
                    ps_s = psS.tile([128, 2 * QW], F32, tag="s")
                    for h in range(HPC):
                        nc.tensor.matmul(
                            ps_s[:, h * QW + c0 : (h + 1) * QW],
                            lhsT=qkT[
                                DH * h : DH * (h + 1),
                                1,
                                b * S + 128 * kt : b * S + 128 * (kt + 1),
                            ],
                            rhs=qkT[DH * h : DH * (h + 1), 0, q0 + c0 : q0 + QW],
                            start=True,
                            stop=True,
                        )
                    ex = ex_pool.tile([128, 2 * QW], BF16, tag="ex")
                    exv = ex[:, :].rearrange("p (i x) -> p i x", x=QW)
                    psv = ps_s[:, :].rearrange("p (i x) -> p i x", x=QW)
                    nc.scalar.activation(
                        out=exv[:, :, c0:QW],
                        in_=psv[:, :, c0:QW],
                        func=AF.Exp,
                        scale=1.0 / np.sqrt(DH),
                    )
                    d0 = 128 * kt - qc * QW
                    if d0 >= 0:
                        # diagonal tile: zero exp(s) where k > q (both heads)
                        nc.gpsimd.affine_select(
                            out=exv[:, :, d0 : d0 + 128],
                            in_=exv[:, :, d0 : d0 + 128],
                            compare_op=ALU.is_ge,
                            fill=0.0,
                            base=0,
                            pattern=[[0, 2], [1, 128]],
                            channel_multiplier=-1,
                        )

                    def av(kt=kt, c0=c0, ex=ex):
                        last = kt == n_kt - 1
                        for h in range(HPC):
                            nc.tensor.matmul(
                                pso[h][:, c0:QW],
                                lhsT=vaug[
                                    :,
                                    b * KTB + kt,
                                    (DH + 1) * h : (DH + 1) * (h + 1),
                                ],
                                rhs=ex[:, h * QW + c0 : (h + 1) * QW],
                                start=(kt == 0),
                                stop=last,
                            )

                    if pending_av is not None:
                        pending_av()
                    pending_av = av
                    flush_po()
                pending_av()

                # softmax denominators: per-head broadcast matmul +
                # reciprocal, then normalize into ao
                for h in range(HPC):
                    dnh = dr_pool.tile([1, QW], F32R, tag="dn")
                    nc.vector.tensor_copy(out=dnh, in_=pso[h][DH : DH + 1, :])
                    psb = psM.tile([DH, QW], F32, tag="m")
                    nc.tensor.matmul(
                        psb, lhsT=ones64, rhs=dnh, start=True, stop=True
                    )
                    rdb = dr_pool.tile([DH, QW], F32, tag="rdb")
                    nc.vector.reciprocal(out=rdb, in_=psb)
                    nc.vector.tensor_mul(
                        out=ao[DH * h : DH * (h + 1), :],
                        in0=pso[h][0:DH, :],
                        in1=rdb,
                    )

                # out-projection, interleaved into the next chunk's kt-loop
                for dcp in range(DCH // 2):
                    def step(dcp=dcp, ao=ao, q0=q0):
                        po_sb = po_pool.tile([128, 2, QW], BF16, tag="po")
                        for i in range(2):
                            dc = 2 * dcp + i
                            ps_po = psM.tile([128, QW], F32, tag="m")
                            nc.tensor.matmul(
                                ps_po,
                                lhsT=wo_sb[:, 128 * dc : 128 * (dc + 1)],
                                rhs=ao,
                                start=True,
                                stop=True,
                            )
                            nc.vector.tensor_copy(out=po_sb[:, i, :], in_=ps_po)
                        dc0 = 2 * dcp
                        nc.sync.dma_start(
                            out=pot_d[
                                128 * dc0 : 128 * (dc0 + 2), q0 : q0 + QW
                            ].rearrange("(i p) q -> p i q", p=128),
                            in_=po_sb,
                        )
                    po_pending.append(step)

            # interleaved schedule: chunk (b, qc) is emitted two projection
            # tiles after its last required tile (transposes lag by 2)
            due = {}
            for b in range(B):
                for qc in range(QC):
                    due.setdefault(b * KTB + 4 * qc + 5, []).append((b, qc))
            for t in range(TT):
                emit_tile(t)
                for (b, qc) in due.get(t, ()):
                    emit_chunk(b, qc)
            while trq:
                trq.pop(0)()
            for tt in sorted(k for k in due if k >= TT):
                for (b, qc) in due[tt]:
                    emit_chunk(b, qc)
            while po_pending:
                flush_po()

    _split_drain_waits(nc)
    return nc


_NC_CACHE = {}


def _get_nc(use_bias=False):
    if use_bias not in _NC_CACHE:
        _NC_CACHE[use_bias] = _build(use_bias)
    return _NC_CACHE[use_bias]


def _prep_inputs(x, Wq, bq, Wk, bk, Wv, bv, Wo):
    xT = np.ascontiguousarray(x.reshape(T, D).T).astype(np.float32)  # [D, T]
    # SBUF tile layout: row (128t+p) = x^T[a*128+p, 128t+j] flattened over (a, j)
    xtl = (
        xT.reshape(DCH, 128, TT, 128)
        .transpose(2, 1, 0, 3)
        .reshape(T, D)
        .astype(BF16NP)
    )
    in_maps = []
    for c in range(NCORES):
        sl = slice(CH * c, CH * (c + 1))
        wq_c = np.array(Wq[sl, :], dtype=np.float32)
        bq_c = np.array(bq[sl], dtype=np.float32)
        wk_c = np.array(Wk[sl, :], dtype=np.float32)
        bk_c = np.array(bk[sl], dtype=np.float32)
        # fold the LayerNorm mean-subtraction (a linear map) into W and b
        for h in range(HPC):
            blk = slice(DH * h, DH * (h + 1))
            wq_c[blk, :] -= wq_c[blk, :].mean(axis=0, keepdims=True)
            bq_c[blk] -= bq_c[blk].mean()
            wk_c[blk, :] -= wk_c[blk, :].mean(axis=0, keepdims=True)
            bk_c[blk] -= bk_c[blk].mean()
        wv_c = np.array(Wv[sl, :], dtype=np.float32)
        bv_c = np.array(bv[sl], dtype=np.float32)
        wqkvt = np.ascontiguousarray(
            np.concatenate([wq_c, wk_c, wv_c], axis=0).T
        ).astype(BF16NP)
        bqkv = np.concatenate([bq_c, bk_c, bv_c])[None, :].astype(np.float32)
        wot = np.ascontiguousarray(Wo[:, sl].T).astype(BF16NP)
        in_maps.append({"xtl": xtl, "wqkvt": wqkvt, "bqkv": bqkv, "wot": wot})
    return in_maps


def kernel(x, mask, Wq, bq, Wk, bk, Wv, bv, Wo, bo, _trace=False):
    x = np.asarray(x, dtype=np.float32)
    in_maps = _prep_inputs(
        x,
        np.asarray(Wq),
        np.asarray(bq),
        np.asarray(Wk),
        np.asarray(bk),
        np.asarray(Wv),
        np.asarray(bv),
        np.asarray(Wo),
    )
    use_bias = bool(
        np.any(np.asarray(bq)) or np.any(np.asarray(bk)) or np.any(np.asarray(bv))
    )
    if not use_bias:
        for m in in_maps:
            del m["bqkv"]
    nc = _get_nc(use_bias)
    res = run_bass_kernel_spmd(
        nc, in_maps, core_ids=list(range(NCORES)), trace=_trace
    )
    pot = np.zeros((D, T), np.float64)
    for c in range(NCORES):
        pot += res.results[c]["pot"].astype(np.float64)
    out = pot.T.astype(np.float32) + np.asarray(bo, dtype=np.float32)[None, :]
    out = out.reshape(B, S, D)
    if _trace:
        return out, res
    return out


# revision 16
# speedup vs baseline: 1.0765x; 1.0765x over previous
"""Multi-head attention (QK-LayerNorm, causal) Trainium2 kernel over 8 NeuronCores.

Sharding: tensor-parallel over heads - 2 heads per core. Each core computes
q/k/v projections for its 128 channels, per-head attention for both batches,
and a partial output projection (its 128-channel slice of Wo); the host sums
the 8 partial projections.

Key layout/throughput choices (vs the f32r baseline):
- All DRAM traffic and matmul operands are bf16 (PE rate is identical to
  fp32r at large free-dims, but DMA bytes halve and small-free matmuls avoid
  the fp32r 4x penalty). PSUM accumulation stays f32.
- x is pre-tiled on the host into the exact SBUF layout, so each input tile
  is one DMA of 128 contiguous 2KB rows (4x fewer descriptors).
- Scores for both heads of a (b, qc, kt) step go into one 2-bank PSUM tile
  [128, 1024]; a single Exp drains both, halving ACT access overhead.
- LayerNorm mean-subtraction is folded into the weights on the host; rstd is
  exp(-0.5*ln(var+eps)) (Exp/Ln share an ACT table; Sqrt does not).
- The softmax denominator is the 65th row of attn@v via a ones-column in V.
- Everything is software-pipelined against the in-order engine streams:
  projection tiles interleave with attention chunks (PSUM pools are shared,
  not phase-scoped), q/k transposes trail their tile by two projection
  steps, attn@v trails exp by one kt step, and the out-projection of chunk
  i drains inside the kt-loop of chunk i+1.
- Engine balance: Square/qkT-drain/rstd on ACT, PSUM-draining reductions,
  multiplies and copies on DVE, diagonal masking on Pool (no PSUM port).
"""

import numpy as np
import ml_dtypes

import concourse.bass as bass
import concourse.mybir as mybir
import concourse.tile as tile
from concourse.bass_utils import run_bass_kernel_spmd
from concourse.masks import make_identity

F32 = mybir.dt.float32
F32R = mybir.dt.float32r
BF16 = mybir.dt.bfloat16
BF16NP = ml_dtypes.bfloat16

B, S, D, H = 2, 2048, 1024, 16
DH = D // H          # 64
NCORES = 8
HPC = H // NCORES    # 2 heads per core
CH = HPC * DH        # 128 channels per core
T = B * S            # 4096 tokens
DCH = D // 128       # 8 contraction chunks
TT = T // 128        # 32 token tiles
QW = 512             # q-chunk width
QC = S // QW         # 4 q-chunks per batch
KTB = S // 128       # 16 k-tiles per batch
EPS = 1e-5


def _split_drain_waits(nc):
    """walrus in this env only accepts one sync-wait per instruction;
    hoist extra waits onto preceding single-wait NOPs on the same engine."""
    for f in nc.m.functions:
        for blk in f.blocks:
            new_insts = []
            for inst in blk.instructions:
                si = getattr(inst, "sync_info", None)
                if si is not None and si.on_wait and len(si.on_wait) > 1:
                    waits = list(si.on_wait)
                    for j, w in enumerate(waits[:-1]):
                        new_insts.append(
                            mybir.InstNoOp(
                                name=f"{inst.name}-dwsplit{j}",
                                engine=inst.engine,
                                ins=[],
                                outs=[],
                                sync_info=mybir.SyncInfo(on_wait=[w], on_update=[]),
                            )
                        )
                    si.on_wait = [waits[-1]]
                    inst.sync_info = si
                new_insts.append(inst)
            blk.instructions[:] = new_insts


def _build(use_bias=False):
    nc = bass.Bass("TRN2", target_bir_lowering=False, debug=False)

    # x pre-tiled on host: row (128*t + p) holds x^T[(a*128+p), 128*t:128*(t+1)]
    # flattened over a, i.e. the SBUF tile layout for token-tile t.
    xtl_d = nc.dram_tensor("xtl", [T, D], BF16, kind="ExternalInput")
    wqkvt_d = nc.dram_tensor("wqkvt", [D, 3 * CH], BF16, kind="ExternalInput")
    bqkv_d = (
        nc.dram_tensor("bqkv", [1, 3 * CH], F32, kind="ExternalInput")
        if use_bias
        else None
    )
    wot_d = nc.dram_tensor("wot", [CH, D], BF16, kind="ExternalInput")
    pot_d = nc.dram_tensor("pot", [D, T], BF16, kind="ExternalOutput")

    AF = mybir.ActivationFunctionType
    ALU = mybir.AluOpType

    with tile.TileContext(nc) as tc:
        with (
            tc.tile_pool(name="const", bufs=1) as const_pool,
            tc.tile_pool(name="big", bufs=1) as big,
            tc.tile_pool(name="xt", bufs=6) as xpool,
            tc.tile_pool(name="sq", bufs=3) as sq_pool,
            tc.tile_pool(name="ln", bufs=4) as ln_pool,
            tc.tile_pool(name="qln", bufs=5) as qln_pool,
            tc.tile_pool(name="qkv", bufs=3) as qkv_pool,
            tc.tile_pool(name="ex", bufs=4) as ex_pool,
            tc.tile_pool(name="ao", bufs=3) as ao_pool,
            tc.tile_pool(name="dr", bufs=3) as dr_pool,
            tc.tile_pool(name="po", bufs=6) as po_pool,
        ):
            ident_f = const_pool.tile([128, 128], F32)
            make_identity(nc, ident_f)
            ident = const_pool.tile([128, 128], BF16)
            nc.vector.tensor_copy(out=ident, in_=ident_f)

            epscol = const_pool.tile([128, 1], F32)
            nc.vector.memset(epscol, EPS)

            ones64f = const_pool.tile([1, DH], F32)
            nc.vector.memset(ones64f, 1.0)
            ones64 = const_pool.tile([1, DH], F32R)
            nc.vector.tensor_copy(out=ones64, in_=ones64f)

            # prefetch the first two x tiles before the (large) weight loads
            xt_first = []
            for t in range(2):
                xt_sb = xpool.tile([128, DCH, 128], BF16, tag="xt")
                xin = xtl_d[128 * t : 128 * (t + 1), :].rearrange(
                    "p (a j) -> p a j", j=128
                )
                for part in range(4):
                    nc.sync.dma_start(
                        out=xt_sb[:, 2 * part : 2 * part + 2, :],
                        in_=xin[:, 2 * part : 2 * part + 2, :],
                    )
                xt_first.append(xt_sb)
            wqkv_sb = const_pool.tile([128, DCH, 3 * CH], BF16)
            for d in range(DCH):
                nc.sync.dma_start(
                    out=wqkv_sb[:, d, :],
                    in_=wqkvt_d[128 * d : 128 * (d + 1), :],
                )
            wo_sb = const_pool.tile([128, D], BF16)
            nc.sync.dma_start(out=wo_sb, in_=wot_d[:, :])
            if use_bias:
                bias_sb = const_pool.tile([128, 3 * CH], F32)
                nc.sync.dma_start(
                    out=bias_sb, in_=bqkv_d[0:1, :].to_broadcast([128, 3 * CH])
                )

            # persistent activations
            qkT = big.tile([128, 2, T], BF16)     # [:,0,:]=q^T  [:,1,:]=k^T
            vaug = big.tile([128, TT, 2 * (DH + 1)], BF16)
            ones32 = const_pool.tile([128, TT, 1], BF16)
            nc.vector.memset(ones32, 1.0)
            for h in range(HPC):
                oc = (DH + 1) * h + DH
                nc.vector.tensor_copy(out=vaug[:, :, oc : oc + 1], in_=ones32)

            trq = []       # lagged transpose+drain closures, one per tile
            po_pending = []  # lagged out-projection steps

            def flush_po():
                if po_pending:
                    po_pending.pop(0)()

            def emit_tile(t):
                if t < 2:
                    xt_sb = xt_first[t]
                else:
                    xt_sb = xpool.tile([128, DCH, 128], BF16, tag="xt")
                    nc.sync.dma_start(
                        out=xt_sb,
                        in_=xtl_d[128 * t : 128 * (t + 1), :].rearrange(
                            "p (a j) -> p a j", j=128
                        ),
                    )
                ps = psA.tile([128, 3 * CH], F32, tag="a")
                for d in range(DCH):
                    nc.tensor.matmul(
                        ps,
                        lhsT=xt_sb[:, d, :],
                        rhs=wqkv_sb[:, d, :],
                        start=(d == 0),
                        stop=(d == DCH - 1),
                    )
                if use_bias:
                    qkv = qkv_pool.tile([128, 3 * CH], F32, tag="qkv")
                    nc.vector.tensor_add(out=qkv, in0=ps, in1=bias_sb)
                    src = qkv
                else:
                    src = ps

                # rstd = exp(-0.5*ln(mean(q'^2) + eps)) per (token, head)
                sq = sq_pool.tile([128, 2 * CH], BF16, tag="sq")
                nc.scalar.activation(out=sq, in_=src[:, 0 : 2 * CH], func=AF.Square)
                ssum = ln_pool.tile([128, 4], F32, tag="ssum")
                nc.vector.reduce_sum(
                    out=ssum,
                    in_=sq.rearrange("p (g x) -> p g x", x=DH),
                    axis=mybir.AxisListType.X,
                )
                lnv = ln_pool.tile([128, 4], F32, tag="lnv")
                nc.scalar.activation(
                    out=lnv, in_=ssum, func=AF.Ln, scale=1.0 / DH,
                    bias=epscol[:, :],
                )
                rstd = ln_pool.tile([128, 4], F32, tag="rstd")
                nc.scalar.activation(out=rstd, in_=lnv, func=AF.Exp, scale=-0.5)

                qln = qln_pool.tile([128, 2 * CH], BF16, tag="qln")
                rstd_ap = rstd[:, :]
                rstd_b = bass.AP(
                    tensor=rstd_ap.tensor,
                    offset=rstd_ap.offset,
                    ap=rstd_ap.ap + [[0, DH]],
                )
                nc.vector.tensor_mul(
                    out=qln.rearrange("p (g x) -> p g x", x=DH),
                    in0=src[:, 0 : 2 * CH].rearrange("p (g x) -> p g x", x=DH),
                    in1=rstd_b,
                )
                nc.vector.tensor_copy(
                    out=vaug[:, t, :].rearrange("p (h x) -> p h x", x=DH + 1)[
                        :, :, 0:DH
                    ],
                    in_=src[:, 2 * CH : 3 * CH].rearrange("p (h x) -> p h x", x=DH),
                )

                def transp(t=t, qln=qln):
                    pst = psTR.tile([128, 256], BF16, tag="t")
                    nc.tensor.transpose(pst[:, 0:128], qln[:, 0:CH], ident)
                    nc.tensor.transpose(pst[:, 128:256], qln[:, CH : 2 * CH], ident)
                    nc.scalar.copy(
                        out=qkT[:, :, 128 * t : 128 * (t + 1)],
                        in_=pst[:, :].rearrange("p (i x) -> p i x", x=128),
                    )

                trq.append(transp)
                if len(trq) > 2:
                    trq.pop(0)()

            def emit_chunk(b, qc):
                q0 = b * S + qc * QW
                n_kt = (qc + 1) * (QW // 128)
                ao = ao_pool.tile([128, QW], BF16, tag="ao")
                pso0 = psO.tile([DH + 1, QW], F32, tag="o")
                pso1 = psO.tile([DH + 1, QW], F32, tag="o")
                pso = [pso0, pso1]
                avq = []
                for kt in range(n_kt):
                    c0 = max(0, 128 * kt - qc * QW)
                    ps_s = psS.tile([128, 2 * QW], F32, tag="s")
                    for h in range(HPC):
                        nc.tensor.matmul(
                            ps_s[:, h * QW + c0 : (h + 1) * QW],
                            lhsT=qkT[
                                DH * h : DH * (h + 1),
                                1,
                                b * S + 128 * kt : b * S + 128 * (kt + 1),
                            ],
                            rhs=qkT[DH * h : DH * (h + 1), 0, q0 + c0 : q0 + QW],
                            start=True,
                            stop=True,
                        )
                    ex = ex_pool.tile([128, 2 * QW], BF16, tag="ex")
                    exv = ex[:, :].rearrange("p (i x) -> p i x", x=QW)
                    psv = ps_s[:, :].rearrange("p (i x) -> p i x", x=QW)
                    nc.scalar.activation(
                        out=exv[:, :, c0:QW],
                        in_=psv[:, :, c0:QW],
                        func=AF.Exp,
                        scale=1.0 / np.sqrt(DH),
                    )
                    d0 = 128 * kt - qc * QW
                    if d0 >= 0:
                        # diagonal tile: zero exp(s) where k > q (both heads)
                        nc.gpsimd.affine_select(
                            out=exv[:, :, d0 : d0 + 128],
                            in_=exv[:, :, d0 : d0 + 128],
                            compare_op=ALU.is_ge,
                            fill=0.0,
                            base=0,
                            pattern=[[0, 2], [1, 128]],
                            channel_multiplier=-1,
                        )

                    def av(kt=kt, c0=c0, ex=ex):
                        last = kt == n_kt - 1
                        for h in range(HPC):
                            nc.tensor.matmul(
                                pso[h][:, c0:QW],
                                lhsT=vaug[
                                    :,
                                    b * KTB + kt,
                                    (DH + 1) * h : (DH + 1) * (h + 1),
                                ],
                                rhs=ex[:, h * QW + c0 : (h + 1) * QW],
                                start=(kt == 0),
                                stop=last,
                            )

                    avq.append(av)
                    if len(avq) > 2:
                        avq.pop(0)()
                    flush_po()
                while avq:
                    avq.pop(0)()

                # softmax denominators: per-head broadcast matmul +
                # reciprocal, then normalize into ao
                for h in range(HPC):
                    dnh = dr_pool.tile([1, QW], F32R, tag="dn")
                    nc.vector.tensor_copy(out=dnh, in_=pso[h][DH : DH + 1, :])
                    psb = psS.tile([DH, QW], F32, tag="s")
                    nc.tensor.matmul(
                        psb, lhsT=ones64, rhs=dnh, start=True, stop=True
                    )
                    rdb = dr_pool.tile([DH, QW], F32, tag="rdb")
                    nc.vector.reciprocal(out=rdb, in_=psb)
                    nc.vector.tensor_mul(
                        out=ao[DH * h : DH * (h + 1), :],
                        in0=pso[h][0:DH, :],
                        in1=rdb,
                    )

                # out-projection, interleaved into the next chunk's kt-loop
                for dcp in range(DCH // 2):
                    def step(dcp=dcp, ao=ao, q0=q0):
                        po_sb = po_pool.tile([128, 2, QW], BF16, tag="po")
                        for i in range(2):
                            dc = 2 * dcp + i
                            ps_po = psS.tile([128, QW], F32, tag="s")
                            nc.tensor.matmul(
                                ps_po,
                                lhsT=wo_sb[:, 128 * dc : 128 * (dc + 1)],
                                rhs=ao,
                                start=True,
                                stop=True,
                            )
                            nc.vector.tensor_copy(out=po_sb[:, i, :], in_=ps_po)
                        dc0 = 2 * dcp
                        nc.sync.dma_start(
                            out=pot_d[
                                128 * dc0 : 128 * (dc0 + 2), q0 : q0 + QW
                            ].rearrange("(i p) q -> p i q", p=128),
                            in_=po_sb,
                        )
                    po_pending.append(step)

            # ---- Phase 1: projection tiles (transposes lag by 2) ----
            psA = tc.alloc_tile_pool(name="psA", bufs=5, space="PSUM")
            psTR = tc.alloc_tile_pool(name="psTR", bufs=3, space="PSUM")
            for t in range(TT):
                emit_tile(t)
            while trq:
                trq.pop(0)()
            psTR.release()
            psA.release()

            # ---- Phase 2: attention chunks (attn@v lags exp by 2) ----
            psS = tc.alloc_tile_pool(name="psS", bufs=3, space="PSUM")
            psO = tc.alloc_tile_pool(name="psO", bufs=2, space="PSUM")
            for b in range(B):
                for qc in range(QC):
                    emit_chunk(b, qc)
            while po_pending:
                flush_po()
            psO.release()
            psS.release()

    _split_drain_waits(nc)
    return nc


_NC_CACHE = {}


def _get_nc(use_bias=False):
    if use_bias not in _NC_CACHE:
        _NC_CACHE[use_bias] = _build(use_bias)
    return _NC_CACHE[use_bias]


def _prep_inputs(x, Wq, bq, Wk, bk, Wv, bv, Wo):
    xT = np.ascontiguousarray(x.reshape(T, D).T).astype(np.float32)  # [D, T]
    # SBUF tile layout: row (128t+p) = x^T[a*128+p, 128t+j] flattened over (a, j)
    xtl = (
        xT.reshape(DCH, 128, TT, 128)
        .transpose(2, 1, 0, 3)
        .reshape(T, D)
        .astype(BF16NP)
    )
    in_maps = []
    for c in range(NCORES):
        sl = slice(CH * c, CH * (c + 1))
        wq_c = np.array(Wq[sl, :], dtype=np.float32)
        bq_c = np.array(bq[sl], dtype=np.float32)
        wk_c = np.array(Wk[sl, :], dtype=np.float32)
        bk_c = np.array(bk[sl], dtype=np.float32)
        # fold the LayerNorm mean-subtraction (a linear map) into W and b
        for h in range(HPC):
            blk = slice(DH * h, DH * (h + 1))
            wq_c[blk, :] -= wq_c[blk, :].mean(axis=0, keepdims=True)
            bq_c[blk] -= bq_c[blk].mean()
            wk_c[blk, :] -= wk_c[blk, :].mean(axis=0, keepdims=True)
            bk_c[blk] -= bk_c[blk].mean()
        wv_c = np.array(Wv[sl, :], dtype=np.float32)
        bv_c = np.array(bv[sl], dtype=np.float32)
        wqkvt = np.ascontiguousarray(
            np.concatenate([wq_c, wk_c, wv_c], axis=0).T
        ).astype(BF16NP)
        bqkv = np.concatenate([bq_c, bk_c, bv_c])[None, :].astype(np.float32)
        wot = np.ascontiguousarray(Wo[:, sl].T).astype(BF16NP)
        in_maps.append({"xtl": xtl, "wqkvt": wqkvt, "bqkv": bqkv, "wot": wot})
    return in_maps


def kernel(x, mask, Wq, bq, Wk, bk, Wv, bv, Wo, bo, _trace=False):
    x = np.asarray(x, dtype=np.float32)
    in_maps = _prep_inputs(
        x,
        np.asarray(Wq),
        np.asarray(bq),
        np.asarray(Wk),
        np.asarray(bk),
        np.asarray(Wv),
        np.asarray(bv),
        np.asarray(Wo),
    )
    use_bias = bool(
        np.any(np.asarray(bq)) or np.any(np.asarray(bk)) or np.any(np.asarray(bv))
    )
    if not use_bias:
        for m in in_maps:
            del m["bqkv"]
    nc = _get_nc(use_bias)
    res = run_bass_kernel_spmd(
        nc, in_maps, core_ids=list(range(NCORES)), trace=_trace
    )
    pot = np.zeros((D, T), np.float64)
    for c in range(NCORES):
        pot += res.results[c]["pot"].astype(np.float64)
    out = pot.T.astype(np.float32) + np.asarray(bo, dtype=np.float32)[None, :]
    out = out.reshape(B, S, D)
    if _trace:
        return out, res
    return out


# revision 20
# speedup vs baseline: 1.1476x; 1.0660x over previous
"""Multi-head attention (QK-LayerNorm, causal) Trainium2 kernel over 8 NeuronCores.

Sharding: tensor-parallel over heads - 2 heads per core. Each core computes
q/k/v projections for its 128 channels, per-head attention for both batches,
and a partial output projection (its 128-channel slice of Wo); the host sums
the 8 partial projections.

Key layout/throughput choices (vs the f32r baseline):
- All DRAM traffic and matmul operands are bf16 (PE rate is identical to
  fp32r at large free-dims, but DMA bytes halve and small-free matmuls avoid
  the fp32r 4x penalty). PSUM accumulation stays f32.
- x is pre-tiled on the host into the exact SBUF layout, so each input tile
  is one DMA of 128 contiguous 2KB rows (4x fewer descriptors).
- Scores for both heads of a (b, qc, kt) step go into one 2-bank PSUM tile
  [128, 1024]; a single Exp drains both, halving ACT access overhead.
- LayerNorm mean-subtraction is folded into the weights on the host; rstd is
  exp(-0.5*ln(var+eps)) (Exp/Ln share an ACT table; Sqrt does not).
- The softmax denominator is the 65th row of attn@v via a ones-column in V.
- Everything is software-pipelined against the in-order engine streams:
  projection tiles interleave with attention chunks (PSUM pools are shared,
  not phase-scoped), q/k transposes trail their tile by two projection
  steps, attn@v trails exp by one kt step, and the out-projection of chunk
  i drains inside the kt-loop of chunk i+1.
- Engine balance: Square/qkT-drain/rstd on ACT, PSUM-draining reductions,
  multiplies and copies on DVE, diagonal masking on Pool (no PSUM port).
"""

import numpy as np
import ml_dtypes

import concourse.bass as bass
import concourse.mybir as mybir
import concourse.tile as tile
from concourse.bass_utils import run_bass_kernel_spmd
from concourse.masks import make_identity

F32 = mybir.dt.float32
F32R = mybir.dt.float32r
BF16 = mybir.dt.bfloat16
BF16NP = ml_dtypes.bfloat16

B, S, D, H = 2, 2048, 1024, 16
DH = D // H          # 64
NCORES = 8
HPC = H // NCORES    # 2 heads per core
CH = HPC * DH        # 128 channels per core
T = B * S            # 4096 tokens
DCH = D // 128       # 8 contraction chunks
TT = T // 128        # 32 token tiles
QW = 512             # q-chunk width
QC = S // QW         # 4 q-chunks per batch
KTB = S // 128       # 16 k-tiles per batch
EPS = 1e-5


def _split_drain_waits(nc):
    """walrus in this env only accepts one sync-wait per instruction;
    hoist extra waits onto preceding single-wait NOPs on the same engine."""
    for f in nc.m.functions:
        for blk in f.blocks:
            new_insts = []
            for inst in blk.instructions:
                si = getattr(inst, "sync_info", None)
                if si is not None and si.on_wait and len(si.on_wait) > 1:
                    waits = list(si.on_wait)
                    for j, w in enumerate(waits[:-1]):
                        new_insts.append(
                            mybir.InstNoOp(
                                name=f"{inst.name}-dwsplit{j}",
                                engine=inst.engine,
                                ins=[],
                                outs=[],
                                sync_info=mybir.SyncInfo(on_wait=[w], on_update=[]),
                            )
                        )
                    si.on_wait = [waits[-1]]
                    inst.sync_info = si
                new_insts.append(inst)
            blk.instructions[:] = new_insts


def _build(use_bias=False):
    nc = bass.Bass("TRN2", target_bir_lowering=False, debug=False)

    # x pre-tiled on host: row (128*t + p) holds x^T[(a*128+p), 128*t:128*(t+1)]
    # flattened over a, i.e. the SBUF tile layout for token-tile t.
    xtl_d = nc.dram_tensor("xtl", [T, D], BF16, kind="ExternalInput")
    wqkvt_d = nc.dram_tensor("wqkvt", [D, 3 * CH], BF16, kind="ExternalInput")
    bqkv_d = (
        nc.dram_tensor("bqkv", [1, 3 * CH], F32, kind="ExternalInput")
        if use_bias
        else None
    )
    wot_d = nc.dram_tensor("wot", [CH, D], BF16, kind="ExternalInput")
    pot_d = nc.dram_tensor("pot", [D, T], BF16, kind="ExternalOutput")

    AF = mybir.ActivationFunctionType
    ALU = mybir.AluOpType

    with tile.TileContext(nc) as tc:
        with (
            tc.tile_pool(name="const", bufs=1) as const_pool,
            tc.tile_pool(name="big", bufs=1) as big,
            tc.tile_pool(name="xt", bufs=6) as xpool,
            tc.tile_pool(name="sq", bufs=3) as sq_pool,
            tc.tile_pool(name="ln", bufs=4) as ln_pool,
            tc.tile_pool(name="qln", bufs=5) as qln_pool,
            tc.tile_pool(name="qkv", bufs=3) as qkv_pool,
            tc.tile_pool(name="ex", bufs=5) as ex_pool,
            tc.tile_pool(name="ao", bufs=3) as ao_pool,
            tc.tile_pool(name="dr", bufs=3) as dr_pool,
            tc.tile_pool(name="po", bufs=6) as po_pool,
        ):
            ident_f = const_pool.tile([128, 128], F32)
            make_identity(nc, ident_f)
            ident = const_pool.tile([128, 128], BF16)
            nc.vector.tensor_copy(out=ident, in_=ident_f)

            epscol = const_pool.tile([128, 1], F32)
            nc.vector.memset(epscol, EPS)

            ones64f = const_pool.tile([1, DH], F32)
            nc.vector.memset(ones64f, 1.0)
            ones64 = const_pool.tile([1, DH], F32R)
            nc.vector.tensor_copy(out=ones64, in_=ones64f)

            # first weight chunk and first x tiles ahead of the bulk loads
            wqkv_sb = const_pool.tile([128, DCH, 3 * CH], BF16)
            nc.sync.dma_start(out=wqkv_sb[:, 0, :], in_=wqkvt_d[0:128, :])
            xt_first = []
            for t in range(2):
                xt_sb = xpool.tile([128, DCH, 128], BF16, tag="xt")
                nc.sync.dma_start(
                    out=xt_sb,
                    in_=xtl_d[128 * t : 128 * (t + 1), :].rearrange(
                        "p (a j) -> p a j", j=128
                    ),
                )
                xt_first.append(xt_sb)
            for d in range(1, DCH):
                nc.sync.dma_start(
                    out=wqkv_sb[:, d, :],
                    in_=wqkvt_d[128 * d : 128 * (d + 1), :],
                )
            wo_sb = const_pool.tile([128, D], BF16)
            nc.sync.dma_start(out=wo_sb, in_=wot_d[:, :])
            if use_bias:
                bias_sb = const_pool.tile([128, 3 * CH], F32)
                nc.sync.dma_start(
                    out=bias_sb, in_=bqkv_d[0:1, :].to_broadcast([128, 3 * CH])
                )

            # persistent activations
            qkT = big.tile([128, 2, T], BF16)     # [:,0,:]=q^T  [:,1,:]=k^T
            vaug = big.tile([128, TT, 2 * (DH + 1)], BF16)
            ones32 = const_pool.tile([128, TT, 1], BF16)
            nc.vector.memset(ones32, 1.0)
            for h in range(HPC):
                oc = (DH + 1) * h + DH
                nc.vector.tensor_copy(out=vaug[:, :, oc : oc + 1], in_=ones32)

            trq = []        # lagged transpose closures, one per tile
            pending_d = []  # deferred attn@v/denominator work from prior chunk
            pending_po = []  # deferred out-projection steps

            def flush_pending():
                # drain items must all be emitted before the new chunk's
                # attn@v writes recycle their PSUM slots, and before any
                # out-projection step that reads the ao they produce
                # (emission-order dependency tracking), so they get priority
                # at a 4/kt rate and out-projection only flushes after them
                for _ in range(4):
                    if not pending_d:
                        break
                    pending_d.pop(0)()
                if not pending_d and pending_po:
                    pending_po.pop(0)()

            def emit_tile(t):
                if t < 2:
                    xt_sb = xt_first[t]
                else:
                    xt_sb = xpool.tile([128, DCH, 128], BF16, tag="xt")
                    nc.sync.dma_start(
                        out=xt_sb,
                        in_=xtl_d[128 * t : 128 * (t + 1), :].rearrange(
                            "p (a j) -> p a j", j=128
                        ),
                    )
                ps = psA.tile([128, 3 * CH], F32, tag="a")
                for d in range(DCH):
                    nc.tensor.matmul(
                        ps,
                        lhsT=xt_sb[:, d, :],
                        rhs=wqkv_sb[:, d, :],
                        start=(d == 0),
                        stop=(d == DCH - 1),
                    )
                if use_bias:
                    qkv = qkv_pool.tile([128, 3 * CH], F32, tag="qkv")
                    nc.vector.tensor_add(out=qkv, in0=ps, in1=bias_sb)
                    src = qkv
                else:
                    src = ps

                # rstd = exp(-0.5*ln(mean(q'^2) + eps)) per (token, head)
                sq = sq_pool.tile([128, 2 * CH], BF16, tag="sq")
                nc.scalar.activation(out=sq, in_=src[:, 0 : 2 * CH], func=AF.Square)
                ssum = ln_pool.tile([128, 4], F32, tag="ssum")
                nc.vector.reduce_sum(
                    out=ssum,
                    in_=sq.rearrange("p (g x) -> p g x", x=DH),
                    axis=mybir.AxisListType.X,
                )
                lnv = ln_pool.tile([128, 4], F32, tag="lnv")
                nc.scalar.activation(
                    out=lnv, in_=ssum, func=AF.Ln, scale=1.0 / DH,
                    bias=epscol[:, :],
                )
                rstd = ln_pool.tile([128, 4], F32, tag="rstd")
                nc.scalar.activation(out=rstd, in_=lnv, func=AF.Exp, scale=-0.5)

                qln = qln_pool.tile([128, 2 * CH], BF16, tag="qln")
                rstd_ap = rstd[:, :]
                rstd_b = bass.AP(
                    tensor=rstd_ap.tensor,
                    offset=rstd_ap.offset,
                    ap=rstd_ap.ap + [[0, DH]],
                )
                nc.vector.tensor_mul(
                    out=qln.rearrange("p (g x) -> p g x", x=DH),
                    in0=src[:, 0 : 2 * CH].rearrange("p (g x) -> p g x", x=DH),
                    in1=rstd_b,
                )
                nc.vector.tensor_copy(
                    out=vaug[:, t, :].rearrange("p (h x) -> p h x", x=DH + 1)[
                        :, :, 0:DH
                    ],
                    in_=src[:, 2 * CH : 3 * CH].rearrange("p (h x) -> p h x", x=DH),
                )

                def transp(t=t, qln=qln):
                    pst = psTR.tile([128, 256], BF16, tag="t")
                    nc.tensor.transpose(pst[:, 0:128], qln[:, 0:CH], ident)
                    nc.tensor.transpose(pst[:, 128:256], qln[:, CH : 2 * CH], ident)
                    nc.scalar.copy(
                        out=qkT[:, :, 128 * t : 128 * (t + 1)],
                        in_=pst[:, :].rearrange("p (i x) -> p i x", x=128),
                    )

                trq.append(transp)
                if len(trq) > 2:
                    trq.pop(0)()

            def emit_chunk(b, qc):
                q0 = b * S + qc * QW
                n_kt = (qc + 1) * (QW // 128)
                ao = ao_pool.tile([128, QW], BF16, tag="ao")
                pso0 = psO.tile([DH + 1, QW], F32, tag="o")
                pso1 = psO.tile([DH + 1, QW], F32, tag="o")
                pso = [pso0, pso1]
                avq = []
                for kt in range(n_kt):
                    c0 = max(0, 128 * kt - qc * QW)
                    ps_s = psS.tile([128, 2 * QW], F32, tag="s")
                    for h in range(HPC):
                        nc.tensor.matmul(
                            ps_s[:, h * QW + c0 : (h + 1) * QW],
                            lhsT=qkT[
                                DH * h : DH * (h + 1),
                                1,
                                b * S + 128 * kt : b * S + 128 * (kt + 1),
                            ],
                            rhs=qkT[DH * h : DH * (h + 1), 0, q0 + c0 : q0 + QW],
                            start=True,
                            stop=True,
                        )
                    ex = ex_pool.tile([128, 2 * QW], BF16, tag="ex")
                    exv = ex[:, :].rearrange("p (i x) -> p i x", x=QW)
                    psv = ps_s[:, :].rearrange("p (i x) -> p i x", x=QW)
                    nc.scalar.activation(
                        out=exv[:, :, c0:QW],
                        in_=psv[:, :, c0:QW],
                        func=AF.Exp,
                        scale=1.0 / np.sqrt(DH),
                    )
                    d0 = 128 * kt - qc * QW
                    if d0 >= 0:
                        # diagonal tile: zero exp(s) where k > q (both heads)
                        nc.gpsimd.affine_select(
                            out=exv[:, :, d0 : d0 + 128],
                            in_=exv[:, :, d0 : d0 + 128],
                            compare_op=ALU.is_ge,
                            fill=0.0,
                            base=0,
                            pattern=[[0, 2], [1, 128]],
                            channel_multiplier=-1,
                        )

                    def av(kt=kt, c0=c0, ex=ex):
                        last = kt == n_kt - 1
                        for h in range(HPC):
                            nc.tensor.matmul(
                                pso[h][:, c0:QW],
                                lhsT=vaug[
                                    :,
                                    b * KTB + kt,
                                    (DH + 1) * h : (DH + 1) * (h + 1),
                                ],
                                rhs=ex[:, h * QW + c0 : (h + 1) * QW],
                                start=(kt == 0),
                                stop=last,
                            )

                    flush_pending()
                    avq.append(av)
                    if len(avq) > 2:
                        avq.pop(0)()

                # defer the remaining attn@v, the softmax-denominator drain
                # and the out-projection into the following chunk's kt-loop
                for fn in avq:
                    pending_d.append(fn)
                avq = []

                state = {}

                def mk_dn(h, pso=pso, state=state):
                    def f(fast=False):
                        dnh = dr_pool.tile([1, QW], F32R, tag="dn")
                        nc.vector.tensor_copy(
                            out=dnh, in_=pso[h][DH : DH + 1, :]
                        )
                        state["dn%d" % h] = dnh
                    return f

                def mk_psbr(h, state=state):
                    def f(fast=False):
                        psb = psS.tile([DH, QW], F32, tag="s")
                        nc.tensor.matmul(
                            psb, lhsT=ones64, rhs=state["dn%d" % h],
                            start=True, stop=True,
                        )
                        rdb = dr_pool.tile([DH, QW], F32, tag="rdb")
                        nc.vector.reciprocal(out=rdb, in_=psb)
                        state["rdb%d" % h] = rdb
                    return f

                def mk_mult(h, pso=pso, ao=ao, state=state):
                    def f(fast=False):
                        nc.vector.tensor_mul(
                            out=ao[DH * h : DH * (h + 1), :],
                            in0=pso[h][0:DH, :],
                            in1=state["rdb%d" % h],
                        )
                    return f

                for h in range(HPC):
                    pending_d.append(mk_dn(h))
                for h in range(HPC):
                    pending_d.append(mk_psbr(h))
                for h in range(HPC):
                    pending_d.append(mk_mult(h))

                for dcp in range(DCH // 2):
                    def step(dcp=dcp, ao=ao, q0=q0, fast=False):
                        po_sb = po_pool.tile([128, 2, QW], BF16, tag="po")
                        for i in range(2):
                            dc = 2 * dcp + i
                            ps_po = psS.tile([128, QW], F32, tag="s")
                            nc.tensor.matmul(
                                ps_po,
                                lhsT=wo_sb[:, 128 * dc : 128 * (dc + 1)],
                                rhs=ao,
                                start=True,
                                stop=True,
                            )
                            if fast:
                                # tail: split the drain between DVE and ACT
                                nc.vector.tensor_copy(
                                    out=po_sb[:, i, 0 : QW // 2],
                                    in_=ps_po[:, 0 : QW // 2],
                                )
                                nc.scalar.copy(
                                    out=po_sb[:, i, QW // 2 : QW],
                                    in_=ps_po[:, QW // 2 : QW],
                                )
                            else:
                                nc.vector.tensor_copy(
                                    out=po_sb[:, i, :], in_=ps_po
                                )
                        dc0 = 2 * dcp
                        nc.sync.dma_start(
                            out=pot_d[
                                128 * dc0 : 128 * (dc0 + 2), q0 : q0 + QW
                            ].rearrange("(i p) q -> p i q", p=128),
                            in_=po_sb,
                        )
                    pending_po.append(step)

            # ---- Phase 1: projection tiles (transposes lag by 2) ----
            psA = tc.alloc_tile_pool(name="psA", bufs=5, space="PSUM")
            psTR = tc.alloc_tile_pool(name="psTR", bufs=3, space="PSUM")
            for t in range(TT):
                emit_tile(t)
            while trq:
                trq.pop(0)()
            psTR.release()
            psA.release()

            # ---- Phase 2: attention chunks (attn@v lags exp by 2) ----
            psS = tc.alloc_tile_pool(name="psS", bufs=3, space="PSUM")
            psO = tc.alloc_tile_pool(name="psO", bufs=2, space="PSUM")
            for b in range(B):
                for qc in range(QC):
                    emit_chunk(b, qc)
            while pending_d:
                pending_d.pop(0)()
            while pending_po:
                pending_po.pop(0)(fast=True)
            psO.release()
            psS.release()

    _split_drain_waits(nc)
    return nc


_NC_CACHE = {}


def _get_nc(use_bias=False):
    if use_bias not in _NC_CACHE:
        _NC_CACHE[use_bias] = _build(use_bias)
    return _NC_CACHE[use_bias]


def _prep_inputs(x, Wq, bq, Wk, bk, Wv, bv, Wo):
    xT = np.ascontiguousarray(x.reshape(T, D).T).astype(np.float32)  # [D, T]
    # SBUF tile layout: row (128t+p) = x^T[a*128+p, 128t+j] flattened over (a, j)
    xtl = (
        xT.reshape(DCH, 128, TT, 128)
        .transpose(2, 1, 0, 3)
        .reshape(T, D)
        .astype(BF16NP)
    )
    in_maps = []
    for c in range(NCORES):
        sl = slice(CH * c, CH * (c + 1))
        wq_c = np.array(Wq[sl, :], dtype=np.float32)
        bq_c = np.array(bq[sl], dtype=np.float32)
        wk_c = np.array(Wk[sl, :], dtype=np.float32)
        bk_c = np.array(bk[sl], dtype=np.float32)
        # fold the LayerNorm mean-subtraction (a linear map) into W and b
        for h in range(HPC):
            blk = slice(DH * h, DH * (h + 1))
            wq_c[blk, :] -= wq_c[blk, :].mean(axis=0, keepdims=True)
            bq_c[blk] -= bq_c[blk].mean()
            wk_c[blk, :] -= wk_c[blk, :].mean(axis=0, keepdims=True)
            bk_c[blk] -= bk_c[blk].mean()
        wv_c = np.array(Wv[sl, :], dtype=np.float32)
        bv_c = np.array(bv[sl], dtype=np.float32)
        wqkvt = np.ascontiguousarray(
            np.concatenate([wq_c, wk_c, wv_c], axis=0).T
        ).astype(BF16NP)
        bqkv = np.concatenate([bq_c, bk_c, bv_c])[None, :].astype(np.float32)
        wot = np.ascontiguousarray(Wo[:, sl].T).astype(BF16NP)
        in_maps.append({"xtl": xtl, "wqkvt": wqkvt, "bqkv": bqkv, "wot": wot})
    return in_maps


def kernel(x, mask, Wq, bq, Wk, bk, Wv, bv, Wo, bo, _trace=False):
    x = np.asarray(x, dtype=np.float32)
    in_maps = _prep_inputs(
        x,
        np.asarray(Wq),
        np.asarray(bq),
        np.asarray(Wk),
        np.asarray(bk),
        np.asarray(Wv),
        np.asarray(bv),
        np.asarray(Wo),
    )
    use_bias = bool(
        np.any(np.asarray(bq)) or np.any(np.asarray(bk)) or np.any(np.asarray(bv))
    )
    if not use_bias:
        for m in in_maps:
            del m["bqkv"]
    nc = _get_nc(use_bias)
    res = run_bass_kernel_spmd(
        nc, in_maps, core_ids=list(range(NCORES)), trace=_trace
    )
    pot = np.zeros((D, T), np.float64)
    for c in range(NCORES):
        pot += res.results[c]["pot"].astype(np.float64)
    out = pot.T.astype(np.float32) + np.asarray(bo, dtype=np.float32)[None, :]
    out = out.reshape(B, S, D)
    if _trace:
        return out, res
    return out


# revision 21
# speedup vs baseline: 1.2058x; 1.0506x over previous
"""Multi-head attention (QK-LayerNorm, causal) Trainium2 kernel over 8 NeuronCores.

Sharding: tensor-parallel over heads — 2 heads per core. Each core computes
q/k/v projections for its 128 channels, per-head attention for both batches,
and a partial output projection (its 128-channel slice of Wo); the host sums
the 8 partial projections.

Device-side layout notes:
- All attention matmuls run on transposed scores s[k, q] so no on-chip
  transposes are needed in the attention inner loop; the only PE transposes
  are q/k tiles ([token, ch] -> [ch, token]) after LayerNorm.
- LayerNorm mean-subtraction is folded into the weights on the host (it is a
  linear map), so on device only an RMS-style rstd = 1/sqrt(mean(q'^2)+eps)
  is needed. rstd is computed as exp(-0.5*ln(var+eps)) because Exp and Ln
  live in the same ACT table set (Sqrt does not), avoiding table thrash.
- The softmax denominator is produced by appending a ones-column to V
  (attn@v then yields numerator rows 0..63 and the denominator in row 64).
- Causality: fully-masked key tiles are skipped by loop bounds, partially
  masked (diagonal) tiles zero the upper triangle of exp(s) via affine_select.
"""

import numpy as np
import ml_dtypes

import concourse.bass as bass
import concourse.mybir as mybir
import concourse.tile as tile
from concourse.bass_utils import run_bass_kernel_spmd
from concourse.masks import make_identity

F32 = mybir.dt.float32
F32R = mybir.dt.float32r
BF16 = mybir.dt.bfloat16
BF16NP = ml_dtypes.bfloat16

B, S, D, H = 2, 2048, 1024, 16
DH = D // H          # 64
NCORES = 8
HPC = H // NCORES    # 2 heads per core
CH = HPC * DH        # 128 channels per core
T = B * S            # 4096 tokens
DCH = D // 128       # 8 contraction chunks
TT = T // 128        # 32 token tiles
QW = 512             # q-chunk width
QC = S // QW         # 4 q-chunks per batch
KTB = S // 128       # 16 k-tiles per batch
EPS = 1e-5


def _split_drain_waits(nc):
    """walrus in this env only accepts one sync-wait per instruction;
    hoist extra waits onto preceding single-wait NOPs on the same engine."""
    for f in nc.m.functions:
        for blk in f.blocks:
            new_insts = []
            for inst in blk.instructions:
                si = getattr(inst, "sync_info", None)
                if si is not None and si.on_wait and len(si.on_wait) > 1:
                    waits = list(si.on_wait)
                    for j, w in enumerate(waits[:-1]):
                        new_insts.append(
                            mybir.InstNoOp(
                                name=f"{inst.name}-dwsplit{j}",
                                engine=inst.engine,
                                ins=[],
                                outs=[],
                                sync_info=mybir.SyncInfo(on_wait=[w], on_update=[]),
                            )
                        )
                    si.on_wait = [waits[-1]]
                    inst.sync_info = si
                new_insts.append(inst)
            blk.instructions[:] = new_insts


def _build(use_bias=False, pcfg=(4, 0, 2, 2), sbufs=(4, 3, 6, 3, 3, 4)):
    a_bufs, o_bufs, s_bufs, b_bufs = pcfg
    x_bufs, qkv_bufs, ex_bufs, ao_bufs, nrm_bufs, po_bufs = sbufs
    nc = bass.Bass("TRN2", target_bir_lowering=False, debug=False)

    xt_d = nc.dram_tensor("xt", [D, T], BF16, kind="ExternalInput")
    wqkvt_d = nc.dram_tensor("wqkvt", [D, 3 * CH], BF16, kind="ExternalInput")
    bqkv_d = (
        nc.dram_tensor("bqkv", [1, 3 * CH], F32, kind="ExternalInput")
        if use_bias
        else None
    )
    wot_d = nc.dram_tensor("wot", [CH, D], F32R, kind="ExternalInput")
    pot_d = nc.dram_tensor("pot", [D, T], BF16, kind="ExternalOutput")

    AF = mybir.ActivationFunctionType
    ALU = mybir.AluOpType

    with tile.TileContext(nc) as tc:
        with (
            tc.tile_pool(name="const", bufs=1) as const_pool,
            tc.tile_pool(name="big", bufs=1) as big,
            tc.tile_pool(name="xt", bufs=x_bufs) as xpool,
            tc.tile_pool(name="qkv", bufs=qkv_bufs) as qkvpool,
            tc.tile_pool(name="ln", bufs=4) as lnpool,
            tc.tile_pool(name="expp", bufs=ex_bufs) as exp_pool,
            tc.tile_pool(name="ao", bufs=ao_bufs) as ao_pool,
            tc.tile_pool(name="nrm", bufs=nrm_bufs) as nrm_pool,
            tc.tile_pool(name="po", bufs=po_bufs) as po_pool,
            tc.tile_pool(name="ps_a", bufs=a_bufs, space="PSUM") as ps_a_pool,
            tc.tile_pool(name="ps_b", bufs=b_bufs, space="PSUM") as ps_b_pool,
            tc.tile_pool(name="ps_s", bufs=s_bufs, space="PSUM") as ps_s_pool,
        ):
            ps_o_pool = (
                tc.alloc_tile_pool(name="ps_o", bufs=o_bufs, space="PSUM")
                if o_bufs
                else None
            )
            identity = const_pool.tile([128, 128], F32)
            make_identity(nc, identity)
            ident_r = const_pool.tile([128, 128], F32R)
            nc.vector.tensor_copy(out=ident_r, in_=identity)

            wqkv_sb = const_pool.tile([128, DCH, 3 * CH], BF16)

            def _load_wqkv(d):
                nc.sync.dma_start(
                    out=wqkv_sb[:, d, :],
                    in_=wqkvt_d[128 * d : 128 * (d + 1), :],
                )

            for d in range(DCH):
                _load_wqkv(d)
            if use_bias:
                bias_sb = const_pool.tile([128, 3 * CH], F32)
                nc.sync.dma_start(
                    out=bias_sb, in_=bqkv_d[0:1, :].to_broadcast([128, 3 * CH])
                )

            qT = big.tile([128, T], F32R)
            kT = big.tile([128, T], F32R)
            vaug = big.tile([128, TT, 2 * (DH + 1)], F32R)
            ones64f = const_pool.tile([1, DH], F32)
            nc.vector.memset(ones64f, 1.0)
            ones64r = const_pool.tile([1, DH], F32R)
            nc.vector.tensor_copy(out=ones64r, in_=ones64f)
            ones32 = const_pool.tile([128, TT, 1], F32)
            nc.vector.memset(ones32, 1.0)
            for oc in (DH, 2 * DH + 1):
                nc.vector.tensor_copy(out=vaug[:, :, oc : oc + 1], in_=ones32)

            # ---- Phase 1: q/k/v projection + LN + transposes ----
            for t in range(TT):
                xt_sb = xpool.tile([128, DCH, 128], BF16, tag="xt")
                hd = DCH // 2
                for part in range(2):
                    nc.sync.dma_start(
                        out=xt_sb[:, part * hd : (part + 1) * hd, :],
                        in_=xt_d[:, 128 * t : 128 * (t + 1)]
                        .rearrange("(a p) t -> p a t", p=128)[
                            :, part * hd : (part + 1) * hd, :
                        ],
                    )
                ps = ps_a_pool.tile([128, 3 * CH], F32, tag="a")
                for d in range(DCH):
                    nc.tensor.matmul(
                        ps,
                        lhsT=xt_sb[:, d, :],
                        rhs=wqkv_sb[:, d, :],
                        start=(d == 0),
                        stop=(d == DCH - 1),
                    )
                if use_bias:
                    qkv = qkvpool.tile([128, 3 * CH], F32, tag="qkv")
                    nc.vector.tensor_add(out=qkv, in0=ps, in1=bias_sb)
                    src_qk = qkv
                else:
                    src_qk = ps

                # RMS-style LN on q and k slices (4 groups of 64)
                sq = lnpool.tile([128, 2 * CH], F32, tag="sq")
                nc.scalar.activation(out=sq, in_=src_qk[:, 0 : 2 * CH], func=AF.Square)
                ssum = lnpool.tile([128, 4], F32, tag="ssum")
                nc.vector.reduce_sum(
                    out=ssum,
                    in_=sq.rearrange("p (g x) -> p g x", x=DH),
                    axis=mybir.AxisListType.X,
                )
                vareps = lnpool.tile([128, 4], F32, tag="vareps")
                nc.vector.tensor_scalar(
                    out=vareps,
                    in0=ssum,
                    scalar1=1.0 / DH,
                    scalar2=EPS,
                    op0=ALU.mult,
                    op1=ALU.add,
                )
                lnv = lnpool.tile([128, 4], F32, tag="lnv")
                nc.scalar.activation(out=lnv, in_=vareps, func=AF.Ln)
                rstd = lnpool.tile([128, 4], F32, tag="rstd")
                nc.scalar.activation(out=rstd, in_=lnv, func=AF.Exp, scale=-0.5)
                qln = qkvpool.tile([128, 2 * CH], F32R, tag="qln")
                rstd_ap = rstd[:, :]
                rstd_b = bass.AP(
                    tensor=rstd_ap.tensor,
                    offset=rstd_ap.offset,
                    ap=rstd_ap.ap + [[0, DH]],
                )
                nc.vector.tensor_mul(
                    out=qln.rearrange("p (g x) -> p g x", x=DH),
                    in0=src_qk[:, 0 : 2 * CH].rearrange("p (g x) -> p g x", x=DH),
                    in1=rstd_b,
                )

                for which, dst in ((0, qT), (1, kT)):
                    pst = ps_b_pool.tile([128, 128], F32R, tag="b")
                    nc.tensor.transpose(
                        pst, qln[:, CH * which : CH * (which + 1)], ident_r
                    )
                    if which == 0:
                        nc.scalar.copy(out=dst[:, 128 * t : 128 * (t + 1)], in_=pst)
                    else:
                        nc.vector.tensor_copy(
                            out=dst[:, 128 * t : 128 * (t + 1)], in_=pst
                        )

                for h in range(HPC):
                    nc.vector.tensor_copy(
                        out=vaug[:, t, (DH + 1) * h : (DH + 1) * h + DH],
                        in_=src_qk[:, 2 * CH + DH * h : 2 * CH + DH * (h + 1)],
                    )

            wo_sb = const_pool.tile([128, D], F32R)
            nc.sync.dma_start(out=wo_sb, in_=wot_d[:, :])

            # ---- Phase 2: per-head causal attention + partial out-projection ----
            for b in range(B):
                for qc in range(QC):
                    q0 = b * S + qc * QW
                    n_kt = (qc + 1) * (QW // 128)
                    ao = ao_pool.tile([128, QW], F32R, tag="ao")
                    for h in range(HPC):
                        if o_bufs:
                            ps_o = ps_o_pool.tile([DH + 1, QW], F32, tag="o")
                        else:
                            ps_o = ps_a_pool.tile([DH + 1, QW], F32, tag="a")
                        for kt in range(n_kt):
                            c0 = max(0, kt * 128 - qc * QW)
                            ps_s = ps_s_pool.tile([128, QW], F32, tag="ps_s")
                            nc.tensor.matmul(
                                ps_s[:, c0:QW],
                                lhsT=kT[
                                    DH * h : DH * (h + 1),
                                    b * S + 128 * kt : b * S + 128 * (kt + 1),
                                ],
                                rhs=qT[DH * h : DH * (h + 1), q0 + c0 : q0 + QW],
                                start=True,
                                stop=True,
                            )
                            ex = exp_pool.tile([128, QW], F32R, tag="ex")
                            nc.scalar.activation(
                                out=ex[:, c0:QW],
                                in_=ps_s[:, c0:QW],
                                func=AF.Exp,
                                scale=1.0 / np.sqrt(DH),
                            )
                            d0 = kt * 128 - qc * QW
                            if d0 >= 0:
                                # diagonal tile: zero exp(s) where k > q
                                nc.gpsimd.affine_select(
                                    out=ex[:, d0 : d0 + 128],
                                    in_=ex[:, d0 : d0 + 128],
                                    compare_op=ALU.is_ge,
                                    fill=0.0,
                                    base=0,
                                    pattern=[[1, 128]],
                                    channel_multiplier=-1,
                                )
                            nc.tensor.matmul(
                                ps_o[:, c0:QW],
                                lhsT=vaug[
                                    :,
                                    b * KTB + kt,
                                    (DH + 1) * h : (DH + 1) * (h + 1),
                                ],
                                rhs=ex[:, c0:QW],
                                start=(kt == 0),
                                stop=(kt == n_kt - 1),
                            )
                        dncp = nrm_pool.tile([1, QW], F32R, tag="dncp")
                        nc.vector.tensor_copy(out=dncp, in_=ps_o[DH : DH + 1, :])
                        psb = ps_b_pool.tile([DH, QW], F32, tag="b")
                        nc.tensor.matmul(
                            psb, lhsT=ones64r, rhs=dncp, start=True, stop=True
                        )
                        rdb = nrm_pool.tile([DH, QW], F32, tag="rdb")
                        nc.vector.reciprocal(out=rdb, in_=psb)
                        nc.vector.tensor_mul(
                            out=ao[DH * h : DH * (h + 1), :],
                            in0=ps_o[0:DH, :],
                            in1=rdb,
                        )
                    for dc in range(DCH):
                        ps_po = ps_b_pool.tile([128, QW], F32, tag="b")
                        nc.tensor.matmul(
                            ps_po,
                            lhsT=wo_sb[:, 128 * dc : 128 * (dc + 1)],
                            rhs=ao,
                            start=True,
                            stop=True,
                        )
                        po_sb = po_pool.tile([128, QW], BF16, tag="po")
                        nc.vector.tensor_copy(out=po_sb, in_=ps_po)
                        nc.sync.dma_start(
                            out=pot_d[128 * dc : 128 * (dc + 1), q0 : q0 + QW],
                            in_=po_sb,
                        )
            if ps_o_pool is not None:
                ps_o_pool.release()

    _split_drain_waits(nc)
    return nc


_NC_CACHE = {}


def _get_nc(use_bias=False):
    if use_bias not in _NC_CACHE:
        _NC_CACHE[use_bias] = _build(use_bias)
    return _NC_CACHE[use_bias]


def _prep_inputs(x, Wq, bq, Wk, bk, Wv, bv, Wo):
    xt = np.ascontiguousarray(x.reshape(T, D).T).astype(BF16NP)
    in_maps = []
    for c in range(NCORES):
        sl = slice(CH * c, CH * (c + 1))
        wq_c = np.array(Wq[sl, :], dtype=np.float32)
        bq_c = np.array(bq[sl], dtype=np.float32)
        wk_c = np.array(Wk[sl, :], dtype=np.float32)
        bk_c = np.array(bk[sl], dtype=np.float32)
        # fold the LayerNorm mean-subtraction (a linear map) into W and b
        for h in range(HPC):
            blk = slice(DH * h, DH * (h + 1))
            wq_c[blk, :] -= wq_c[blk, :].mean(axis=0, keepdims=True)
            bq_c[blk] -= bq_c[blk].mean()
            wk_c[blk, :] -= wk_c[blk, :].mean(axis=0, keepdims=True)
            bk_c[blk] -= bk_c[blk].mean()
        wv_c = np.array(Wv[sl, :], dtype=np.float32)
        bv_c = np.array(bv[sl], dtype=np.float32)
        wqkvt = np.ascontiguousarray(
            np.concatenate([wq_c, wk_c, wv_c], axis=0).T
        ).astype(BF16NP)
        bqkv = np.concatenate([bq_c, bk_c, bv_c])[None, :].astype(np.float32)
        wot = np.ascontiguousarray(Wo[:, sl].T).astype(np.float32)
        in_maps.append({"xt": xt, "wqkvt": wqkvt, "bqkv": bqkv, "wot": wot})
    return in_maps


def kernel(x, mask, Wq, bq, Wk, bk, Wv, bv, Wo, bo, _trace=False):
    x = np.asarray(x, dtype=np.float32)
    in_maps = _prep_inputs(
        x,
        np.asarray(Wq),
        np.asarray(bq),
        np.asarray(Wk),
        np.asarray(bk),
        np.asarray(Wv),
        np.asarray(bv),
        np.asarray(Wo),
    )
    use_bias = bool(
        np.any(np.asarray(bq)) or np.any(np.asarray(bk)) or np.any(np.asarray(bv))
    )
    nc = _get_nc(use_bias)
    res = run_bass_kernel_spmd(
        nc, in_maps, core_ids=list(range(NCORES)), trace=_trace
    )
    pot = np.zeros((D, T), np.float64)
    for c in range(NCORES):
        pot += res.results[c]["pot"].astype(np.float64)
    out = pot.T.astype(np.float32) + np.asarray(bo, dtype=np.float32)[None, :]
    out = out.reshape(B, S, D)
    if _trace:
        return out, res
    return out



# revision 22
# speedup vs baseline: 1.2222x; 1.0137x over previous
"""Multi-head attention (QK-LayerNorm, causal) Trainium2 kernel over 8 NeuronCores.

Sharding: tensor-parallel over heads — 2 heads per core. Each core computes
q/k/v projections for its 128 channels, per-head attention for both batches,
and a partial output projection (its 128-channel slice of Wo); the host sums
the 8 partial projections.

Device-side layout notes:
- All attention matmuls run on transposed scores s[k, q] so no on-chip
  transposes are needed in the attention inner loop; the only PE transposes
  are q/k tiles ([token, ch] -> [ch, token]) after LayerNorm.
- LayerNorm mean-subtraction is folded into the weights on the host (it is a
  linear map), so on device only an RMS-style rstd = 1/sqrt(mean(q'^2)+eps)
  is needed. rstd is computed as exp(-0.5*ln(var+eps)) because Exp and Ln
  live in the same ACT table set (Sqrt does not), avoiding table thrash.
- The softmax denominator is produced by appending a ones-column to V
  (attn@v then yields numerator rows 0..63 and the denominator in row 64).
- Causality: fully-masked key tiles are skipped by loop bounds, partially
  masked (diagonal) tiles zero the upper triangle of exp(s) via affine_select.
"""

import numpy as np

import concourse.bass as bass
import concourse.mybir as mybir
import concourse.tile as tile
from concourse.bass_utils import run_bass_kernel_spmd
from concourse.masks import make_identity

F32 = mybir.dt.float32
F32R = mybir.dt.float32r

B, S, D, H = 2, 2048, 1024, 16
DH = D // H          # 64
NCORES = 8
HPC = H // NCORES    # 2 heads per core
CH = HPC * DH        # 128 channels per core
T = B * S            # 4096 tokens
DCH = D // 128       # 8 contraction chunks
TT = T // 128        # 32 token tiles
QW = 512             # q-chunk width
QC = S // QW         # 4 q-chunks per batch
KTB = S // 128       # 16 k-tiles per batch
EPS = 1e-5


def _split_drain_waits(nc):
    """walrus in this env only accepts one sync-wait per instruction;
    hoist extra waits onto preceding single-wait NOPs on the same engine."""
    for f in nc.m.functions:
        for blk in f.blocks:
            new_insts = []
            for inst in blk.instructions:
                si = getattr(inst, "sync_info", None)
                if si is not None and si.on_wait and len(si.on_wait) > 1:
                    waits = list(si.on_wait)
                    for j, w in enumerate(waits[:-1]):
                        new_insts.append(
                            mybir.InstNoOp(
                                name=f"{inst.name}-dwsplit{j}",
                                engine=inst.engine,
                                ins=[],
                                outs=[],
                                sync_info=mybir.SyncInfo(on_wait=[w], on_update=[]),
                            )
                        )
                    si.on_wait = [waits[-1]]
                    inst.sync_info = si
                new_insts.append(inst)
            blk.instructions[:] = new_insts


def _build(use_bias=False, pcfg=(4, 0, 2, 2), sbufs=(4, 3, 6, 3, 3, 4)):
    a_bufs, o_bufs, s_bufs, b_bufs = pcfg
    x_bufs, qkv_bufs, ex_bufs, ao_bufs, nrm_bufs, po_bufs = sbufs
    nc = bass.Bass("TRN2", target_bir_lowering=False, debug=False)

    xt_d = nc.dram_tensor("xt", [D, T], F32R, kind="ExternalInput")
    wqkvt_d = nc.dram_tensor("wqkvt", [D, 3 * CH], F32R, kind="ExternalInput")
    bqkv_d = (
        nc.dram_tensor("bqkv", [1, 3 * CH], F32, kind="ExternalInput")
        if use_bias
        else None
    )
    wot_d = nc.dram_tensor("wot", [CH, D], F32R, kind="ExternalInput")
    pot_d = nc.dram_tensor("pot", [D, T], F32, kind="ExternalOutput")

    AF = mybir.ActivationFunctionType
    ALU = mybir.AluOpType

    with tile.TileContext(nc) as tc:
        with (
            tc.tile_pool(name="const", bufs=1) as const_pool,
            tc.tile_pool(name="big", bufs=1) as big,
            tc.tile_pool(name="xt", bufs=x_bufs) as xpool,
            tc.tile_pool(name="qkv", bufs=qkv_bufs) as qkvpool,
            tc.tile_pool(name="ln", bufs=4) as lnpool,
            tc.tile_pool(name="expp", bufs=ex_bufs) as exp_pool,
            tc.tile_pool(name="ao", bufs=ao_bufs) as ao_pool,
            tc.tile_pool(name="nrm", bufs=nrm_bufs) as nrm_pool,
            tc.tile_pool(name="po", bufs=po_bufs) as po_pool,
            tc.tile_pool(name="ps_a", bufs=a_bufs, space="PSUM") as ps_a_pool,
            tc.tile_pool(name="ps_b", bufs=b_bufs, space="PSUM") as ps_b_pool,
            tc.tile_pool(name="ps_s", bufs=s_bufs, space="PSUM") as ps_s_pool,
        ):
            ps_o_pool = (
                tc.alloc_tile_pool(name="ps_o", bufs=o_bufs, space="PSUM")
                if o_bufs
                else None
            )
            identity = const_pool.tile([128, 128], F32)
            make_identity(nc, identity)
            ident_r = const_pool.tile([128, 128], F32R)
            nc.vector.tensor_copy(out=ident_r, in_=identity)

            wqkv_sb = const_pool.tile([128, DCH, 3 * CH], F32R)

            def _load_wqkv(d):
                nc.sync.dma_start(
                    out=wqkv_sb[:, d, :],
                    in_=wqkvt_d[128 * d : 128 * (d + 1), :],
                )

            for d in range(DCH):
                _load_wqkv(d)
            if use_bias:
                bias_sb = const_pool.tile([128, 3 * CH], F32)
                nc.sync.dma_start(
                    out=bias_sb, in_=bqkv_d[0:1, :].to_broadcast([128, 3 * CH])
                )

            qT = big.tile([128, T], F32R)
            kT = big.tile([128, T], F32R)
            vaug = big.tile([128, TT, 2 * (DH + 1)], F32R)
            ones64f = const_pool.tile([1, DH], F32)
            nc.vector.memset(ones64f, 1.0)
            ones64r = const_pool.tile([1, DH], F32R)
            nc.vector.tensor_copy(out=ones64r, in_=ones64f)
            ones32 = const_pool.tile([128, TT, 1], F32)
            nc.vector.memset(ones32, 1.0)
            for oc in (DH, 2 * DH + 1):
                nc.vector.tensor_copy(out=vaug[:, :, oc : oc + 1], in_=ones32)

            # ---- Phase 1: q/k/v projection + LN + transposes ----
            for t in range(TT):
                xt_sb = xpool.tile([128, DCH, 128], F32R, tag="xt")
                hd = DCH // 2
                for part in range(2):
                    nc.sync.dma_start(
                        out=xt_sb[:, part * hd : (part + 1) * hd, :],
                        in_=xt_d[:, 128 * t : 128 * (t + 1)]
                        .rearrange("(a p) t -> p a t", p=128)[
                            :, part * hd : (part + 1) * hd, :
                        ],
                    )
                ps = ps_a_pool.tile([128, 3 * CH], F32, tag="a")
                for d in range(DCH):
                    nc.tensor.matmul(
                        ps,
                        lhsT=xt_sb[:, d, :],
                        rhs=wqkv_sb[:, d, :],
                        start=(d == 0),
                        stop=(d == DCH - 1),
                    )
                if use_bias:
                    qkv = qkvpool.tile([128, 3 * CH], F32, tag="qkv")
                    nc.vector.tensor_add(out=qkv, in0=ps, in1=bias_sb)
                    src_qk = qkv
                else:
                    src_qk = ps

                # RMS-style LN on q and k slices (4 groups of 64)
                sq = lnpool.tile([128, 2 * CH], F32, tag="sq")
                nc.scalar.activation(out=sq, in_=src_qk[:, 0 : 2 * CH], func=AF.Square)
                ssum = lnpool.tile([128, 4], F32, tag="ssum")
                nc.vector.reduce_sum(
                    out=ssum,
                    in_=sq.rearrange("p (g x) -> p g x", x=DH),
                    axis=mybir.AxisListType.X,
                )
                vareps = lnpool.tile([128, 4], F32, tag="vareps")
                nc.vector.tensor_scalar(
                    out=vareps,
                    in0=ssum,
                    scalar1=1.0 / DH,
                    scalar2=EPS,
                    op0=ALU.mult,
                    op1=ALU.add,
                )
                lnv = lnpool.tile([128, 4], F32, tag="lnv")
                nc.scalar.activation(out=lnv, in_=vareps, func=AF.Ln)
                rstd = lnpool.tile([128, 4], F32, tag="rstd")
                nc.scalar.activation(out=rstd, in_=lnv, func=AF.Exp, scale=-0.5)
                qln = qkvpool.tile([128, 2 * CH], F32R, tag="qln")
                rstd_ap = rstd[:, :]
                rstd_b = bass.AP(
                    tensor=rstd_ap.tensor,
                    offset=rstd_ap.offset,
                    ap=rstd_ap.ap + [[0, DH]],
                )
                nc.vector.tensor_mul(
                    out=qln.rearrange("p (g x) -> p g x", x=DH),
                    in0=src_qk[:, 0 : 2 * CH].rearrange("p (g x) -> p g x", x=DH),
                    in1=rstd_b,
                )

                for which, dst in ((0, qT), (1, kT)):
                    pst = ps_b_pool.tile([128, 128], F32R, tag="b")
                    nc.tensor.transpose(
                        pst, qln[:, CH * which : CH * (which + 1)], ident_r
                    )
                    if which == 0:
                        nc.scalar.copy(out=dst[:, 128 * t : 128 * (t + 1)], in_=pst)
                    else:
                        nc.vector.tensor_copy(
                            out=dst[:, 128 * t : 128 * (t + 1)], in_=pst
                        )

                for h in range(HPC):
                    nc.vector.tensor_copy(
                        out=vaug[:, t, (DH + 1) * h : (DH + 1) * h + DH],
                        in_=src_qk[:, 2 * CH + DH * h : 2 * CH + DH * (h + 1)],
                    )

            wo_sb = const_pool.tile([128, D], F32R)
            nc.sync.dma_start(out=wo_sb, in_=wot_d[:, :])

            # ---- Phase 2: per-head causal attention + partial out-projection ----
            for b in range(B):
                for qc in range(QC):
                    q0 = b * S + qc * QW
                    n_kt = (qc + 1) * (QW // 128)
                    ao = ao_pool.tile([128, QW], F32R, tag="ao")
                    for h in range(HPC):
                        if o_bufs:
                            ps_o = ps_o_pool.tile([DH + 1, QW], F32, tag="o")
                        else:
                            ps_o = ps_a_pool.tile([DH + 1, QW], F32, tag="a")
                        for kt in range(n_kt):
                            c0 = max(0, kt * 128 - qc * QW)
                            ps_s = ps_s_pool.tile([128, QW], F32, tag="ps_s")
                            nc.tensor.matmul(
                                ps_s[:, c0:QW],
                                lhsT=kT[
                                    DH * h : DH * (h + 1),
                                    b * S + 128 * kt : b * S + 128 * (kt + 1),
                                ],
                                rhs=qT[DH * h : DH * (h + 1), q0 + c0 : q0 + QW],
                                start=True,
                                stop=True,
                            )
                            ex = exp_pool.tile([128, QW], F32R, tag="ex")
                            nc.scalar.activation(
                                out=ex[:, c0:QW],
                                in_=ps_s[:, c0:QW],
                                func=AF.Exp,
                                scale=1.0 / np.sqrt(DH),
                            )
                            d0 = kt * 128 - qc * QW
                            if d0 >= 0:
                                # diagonal tile: zero exp(s) where k > q
                                nc.gpsimd.affine_select(
                                    out=ex[:, d0 : d0 + 128],
                                    in_=ex[:, d0 : d0 + 128],
                                    compare_op=ALU.is_ge,
                                    fill=0.0,
                                    base=0,
                                    pattern=[[1, 128]],
                                    channel_multiplier=-1,
                                )
                            nc.tensor.matmul(
                                ps_o[:, c0:QW],
                                lhsT=vaug[
                                    :,
                                    b * KTB + kt,
                                    (DH + 1) * h : (DH + 1) * (h + 1),
                                ],
                                rhs=ex[:, c0:QW],
                                start=(kt == 0),
                                stop=(kt == n_kt - 1),
                            )
                        dncp = nrm_pool.tile([1, QW], F32R, tag="dncp")
                        nc.vector.tensor_copy(out=dncp, in_=ps_o[DH : DH + 1, :])
                        psb = ps_b_pool.tile([DH, QW], F32, tag="b")
                        nc.tensor.matmul(
                            psb, lhsT=ones64r, rhs=dncp, start=True, stop=True
                        )
                        rdb = nrm_pool.tile([DH, QW], F32, tag="rdb")
                        nc.vector.reciprocal(out=rdb, in_=psb)
                        nc.vector.tensor_mul(
                            out=ao[DH * h : DH * (h + 1), :],
                            in0=ps_o[0:DH, :],
                            in1=rdb,
                        )
                    for dc in range(DCH):
                        ps_po = ps_b_pool.tile([128, QW], F32, tag="b")
                        nc.tensor.matmul(
                            ps_po,
                            lhsT=wo_sb[:, 128 * dc : 128 * (dc + 1)],
                            rhs=ao,
                            start=True,
                            stop=True,
                        )
                        po_sb = po_pool.tile([128, QW], F32, tag="po")
                        nc.vector.tensor_copy(out=po_sb, in_=ps_po)
                        nc.sync.dma_start(
                            out=pot_d[128 * dc : 128 * (dc + 1), q0 : q0 + QW],
                            in_=po_sb,
                        )
            if ps_o_pool is not None:
                ps_o_pool.release()

    _split_drain_waits(nc)
    return nc


_NC_CACHE = {}


def _get_nc(use_bias=False):
    if use_bias not in _NC_CACHE:
        _NC_CACHE[use_bias] = _build(use_bias)
    return _NC_CACHE[use_bias]


def _prep_inputs(x, Wq, bq, Wk, bk, Wv, bv, Wo):
    xt = np.ascontiguousarray(x.reshape(T, D).T).astype(np.float32)
    in_maps = []
    for c in range(NCORES):
        sl = slice(CH * c, CH * (c + 1))
        wq_c = np.array(Wq[sl, :], dtype=np.float32)
        bq_c = np.array(bq[sl], dtype=np.float32)
        wk_c = np.array(Wk[sl, :], dtype=np.float32)
        bk_c = np.array(bk[sl], dtype=np.float32)
        # fold the LayerNorm mean-subtraction (a linear map) into W and b
        for h in range(HPC):
            blk = slice(DH * h, DH * (h + 1))
            wq_c[blk, :] -= wq_c[blk, :].mean(axis=0, keepdims=True)
            bq_c[blk] -= bq_c[blk].mean()
            wk_c[blk, :] -= wk_c[blk, :].mean(axis=0, keepdims=True)
            bk_c[blk] -= bk_c[blk].mean()
        wv_c = np.array(Wv[sl, :], dtype=np.float32)
        bv_c = np.array(bv[sl], dtype=np.float32)
        wqkvt = np.ascontiguousarray(
            np.concatenate([wq_c, wk_c, wv_c], axis=0).T
        ).astype(np.float32)
        bqkv = np.concatenate([bq_c, bk_c, bv_c])[None, :].astype(np.float32)
        wot = np.ascontiguousarray(Wo[:, sl].T).astype(np.float32)
        in_maps.append({"xt": xt, "wqkvt": wqkvt, "bqkv": bqkv, "wot": wot})
    return in_maps


def kernel(x, mask, Wq, bq, Wk, bk, Wv, bv, Wo, bo, _trace=False):
    x = np.asarray(x, dtype=np.float32)
    in_maps = _prep_inputs(
        x,
        np.asarray(Wq),
        np.asarray(bq),
        np.asarray(Wk),
        np.asarray(bk),
        np.asarray(Wv),
        np.asarray(bv),
        np.asarray(Wo),
    )
    use_bias = bool(
        np.any(np.asarray(bq)) or np.any(np.asarray(bk)) or np.any(np.asarray(bv))
    )
    nc = _get_nc(use_bias)
    res = run_bass_kernel_spmd(
        nc, in_maps, core_ids=list(range(NCORES)), trace=_trace
    )
    pot = np.zeros((D, T), np.float64)
    for c in range(NCORES):
        pot += res.results[c]["pot"].astype(np.float64)
    out = pot.T.astype(np.float32) + np.asarray(bo, dtype=np.float32)[None, :]
    out = out.reshape(B, S, D)
    if _trace:
        return out, res
    return out

